# revision 17
# baseline (speedup 1.0000x reference)
"""Trainium2 Bass kernel for BatteryMoEFlattenIntraCycleMoELayer.

Computation (reference):
    gates = renorm(top2(softmax(logits) * mask))          # [B, E]
    x = cycle_curve_data.reshape(B, L, 900)
    out[b] = sum_e gates[b,e] * (x[b] @ W[e] + b[e])      # -> bf16 [B, L, 512]

Strategy (bf16, gate-prescaled x, K padded to 1024):
  - Host: compute gates + top-2 routing; build TWO gate-prescaled
    copies of x per sample (xA = gA*x_aug, xB = gB*x_aug, bias row
    included), packed feat-major [B, 128, 8, 128] bf16 (k = sub*128+p,
    zero-padded K 901->1024).  W augmented/padded the same way.
  - Because x carries the gate, both experts' matmuls accumulate into
    ONE PSUM bank per sample: 16 uniform [128,128]x[128,512] bf16
    matmuls -> psum; the combine collapses to a single ACT-engine
    copy/cast psum -> bf16.  One bank/sample makes 8 samples
    k-in-flight possible, which hides the 8.4 MB weight stream during
    phase 1 (k-outer waves of 16 matmuls >= per-tile DMA time).
  - Shard B across 8 cores (64 samples each); routing carried as data
    (per-sample W-slot offsets read into PE registers -> dynamic APs
    on the moving W operand), so one SPMD program serves all cores.
"""

import os
import sys

for _p in ("/opt/trn_rl_repo", "/root/.axon_site/_ro/trn_rl_repo"):
    if os.path.isdir(_p) and _p not in sys.path:
        sys.path.insert(0, _p)

import numpy as np
import ml_dtypes

import concourse.bass as bass
import concourse.mybir as mybir
import concourse.tile as tile
from concourse import bacc
from concourse.bass_utils import run_bass_kernel_spmd
from concourse.bass_values import RuntimeValue

B, L, CURVE_LEN = 512, 128, 300
FEAT = 3 * CURVE_LEN          # 900
FEAT_AUG = FEAT + 1           # 901 (bias row)
K_PAD = 1024                  # zero-padded K: 8 uniform chunks of 128
N_KCH = 8
D_MODEL = 512
NUM_EXPERTS = 8
TOP_K = 2
EPS = 1e-9
N_CORES = 8
S = B // N_CORES              # 64 samples per core

BF16 = ml_dtypes.bfloat16

_CACHE = {}


def _build_nc():
    """Build the SPMD Bass program (routing-independent)."""
    nc = bacc.Bacc(trn_type="TRN2")
    f32 = mybir.dt.float32
    bf16 = mybir.dt.bfloat16
    i32 = mybir.dt.int32

    # gate-prescaled x copies: [S, part, A/B, sub, L] bf16, k = sub*128+part
    xab_h = nc.declare_dram_parameter("xab", [S, 128, 2, N_KCH, L], bf16,
                                      isOutput=False)
    # w per k-chunk: [k, part, expert*512] bf16 (zero-padded rows)
    w_h = nc.declare_dram_parameter(
        "w", [N_KCH, 128, NUM_EXPERTS * D_MODEL], bf16, isOutput=False)
    widx_h = nc.declare_dram_parameter("widx", [1, 2 * S], i32, isOutput=False)
    y_h = nc.declare_dram_parameter("y", [S, L, D_MODEL], bf16, isOutput=True)

    with tile.TileContext(nc) as tc:
        with (
            tc.tile_pool(name="cpool", bufs=1) as cpool,
            tc.tile_pool(name="xpool", bufs=20) as xpool,
            tc.tile_pool(name="opool", bufs=6) as opool,
            tc.tile_pool(name="pspool", bufs=8, space="PSUM") as pspool,
        ):
            widx_sb = cpool.tile([1, 2 * S], i32)
            nc.sync.dma_start(out=widx_sb[:, :], in_=widx_h[:, :])

            w_sb = []
            for k in range(N_KCH):
                wt = cpool.tile([128, NUM_EXPERTS * D_MODEL], bf16,
                                name=f"w_sb_{k}")
                w_sb.append(wt)

            def load_w(k, nsplit=4):
                # column chunks spread across queues; later tiles use
                # fewer chunks to cut Sync-engine trigger serialization
                WCOL = NUM_EXPERTS * D_MODEL // nsplit
                for c in range(nsplit):
                    nc.sync.dma_start(
                        out=w_sb[k][:, c * WCOL: (c + 1) * WCOL],
                        in_=w_h[k, :, c * WCOL: (c + 1) * WCOL],
                    )

            # ring of PE registers for the per-sample W-slot offsets
            NRING = 16
            wregs = [nc.tensor.alloc_register(f"widx_reg{i}")
                     for i in range(NRING)]
            WMAX = (NUM_EXPERTS - 1) * D_MODEL

            def load_x(s):
                # one DMA trigger per sample: both prescaled copies in
                # a single [128, 2, 8, 128] tile (4 KB/partition)
                xT = xpool.tile([128, 2, N_KCH, L], bf16, tag="x",
                                name=f"xab_sb_{s}")
                nc.sync.dma_start(out=xT[:, :, :, :], in_=xab_h[s])
                return xT

            def load_widx(s0):
                # 8 registers <- widx[2*s0 : 2*s0+8] (4 samples) in one load
                regs = [wregs[(2 * s0 + j) % NRING] for j in range(8)]
                nc.tensor.reg_load(regs, widx_sb[0:1, 2 * s0: 2 * s0 + 8])
                return [RuntimeValue(val=r, min_val=0, max_val=WMAX)
                        for r in regs]

            def mm(ps, x_sb, j, rv, k, start, stop):
                nc.tensor.matmul(
                    ps[:, :], x_sb[:, j, k, :],
                    w_sb[k][:, bass.ds(rv, D_MODEL)],
                    start=start, stop=stop,
                )

            def combine(s, ps):
                o_sb = opool.tile([128, D_MODEL], bf16, tag="o", name=f"o_{s}")
                nc.scalar.copy(o_sb[:, :], ps[:, :])
                nc.sync.dma_start(out=y_h[s, :, :], in_=o_sb[:, :])

            def kouter_group(samples, xs, rv_of):
                """k-outer over a group of samples, 1 PSUM bank each."""
                pss = {s: pspool.tile([128, D_MODEL], f32, tag="ps",
                                      name=f"ps_{s}") for s in samples}
                for k in range(N_KCH):
                    for s in samples:
                        rvA, rvB = rv_of[s]
                        mm(pss[s], xs[s], 0, rvA, k,
                           start=(k == 0), stop=False)
                        mm(pss[s], xs[s], 1, rvB, k,
                           start=False, stop=(k == N_KCH - 1))
                for s in samples:
                    combine(s, pss[s])

            # --- startup DMA order: first group's deps first, W stream
            # interleaved with later groups' x tiles ---
            xs = {}
            load_w(0)
            for s in range(0, 4):
                xs[s] = load_x(s)
            load_w(1)
            for s in range(4, 8):
                xs[s] = load_x(s)
            load_w(2)
            for s in range(8, 12):
                xs[s] = load_x(s)
            load_w(3)
            load_w(4)
            for s in range(12, 16):
                xs[s] = load_x(s)
            load_w(5)
            for s in range(16, 20):
                xs[s] = load_x(s)
            load_w(6)
            load_w(7)

            rv_of = {}

            def load_rv(s0s):
                # each 8-reg batch lands in alternating ring halves; the
                # in-order tensor queue makes reuse safe once the prior
                # group's matmuls have been emitted
                for s0 in s0s:
                    rvs = load_widx(s0)
                    for j in range(4):
                        rv_of[s0 + j] = (rvs[2 * j], rvs[2 * j + 1])

            # --- phase 1: k-outer groups sized to hide the W stream ---
            load_rv((0,))
            kouter_group(range(0, 4), xs, rv_of)
            load_rv((4, 8))
            kouter_group(range(4, 12), xs, rv_of)
            load_rv((12, 16))
            kouter_group(range(12, 20), xs, rv_of)
            for s in range(20):
                del xs[s]

            # --- phase 2: steady state, sample-major ---
            P2 = 20
            for s in range(P2, S):
                xT = load_x(s)
                if s % 4 == 0:
                    rvs = load_widx(s)
                    for j in range(4):
                        if s + j < S:
                            rv_of[s + j] = (rvs[2 * j], rvs[2 * j + 1])
                rvA, rvB = rv_of[s]

                ps = pspool.tile([128, D_MODEL], f32, tag="ps",
                                 name=f"ps2_{s}")
                for k in range(N_KCH):
                    mm(ps, xT, 0, rvA, k, start=(k == 0), stop=False)
                    mm(ps, xT, 1, rvB, k, start=False, stop=(k == N_KCH - 1))
                combine(s, ps)

    nc.finalize()  # Bacc: reg graph-coloring + codegen passes, then freeze
    return nc


def _gates_np(logits, moe_masks):
    """Mirror reference _gates in numpy (fp32)."""
    lg = logits.astype(np.float32)
    m = lg.max(axis=1, keepdims=True)
    e = np.exp(lg - m)
    g = e / e.sum(axis=1, keepdims=True)
    g = g * (moe_masks == 1).astype(np.float32)
    # top-2, ties -> lower index first (matches jax.lax.top_k)
    top_idx = np.argsort(-g, axis=1, kind="stable")[:, :TOP_K]
    rows = np.arange(g.shape[0])[:, None]
    gsel = g[rows, top_idx]                                  # [B, 2]
    gsel = gsel / (gsel.sum(axis=1, keepdims=True) + EPS)
    return gsel.astype(np.float32), top_idx.astype(np.int32)


def _pack_x(xs):
    """[B, L, K_PAD] f32 -> [B, 128, sub, L] bf16 with k = sub*128 + p."""
    return np.ascontiguousarray(
        xs.astype(BF16).reshape(B, L, N_KCH, 128).transpose(0, 3, 2, 1))


def _prep_inputs(cycle_curve_data, logits, moe_masks, W, b):
    gsel, top_idx = _gates_np(logits, moe_masks)

    xf = cycle_curve_data.reshape(B, L, FEAT).astype(np.float32, copy=False)
    xq = np.zeros((B, L, K_PAD), np.float32)
    xq[:, :, :FEAT] = xf
    xq[:, :, FEAT] = 1.0
    xa = _pack_x(xq * gsel[:, 0].reshape(B, 1, 1))
    xb = _pack_x(xq * gsel[:, 1].reshape(B, 1, 1))
    xab = np.ascontiguousarray(np.stack([xa, xb], axis=2))  # [B,128,2,8,L]

    w_aug = np.zeros((NUM_EXPERTS, K_PAD, D_MODEL), np.float32)
    w_aug[:, :FEAT, :] = W.astype(np.float32)
    w_aug[:, FEAT, :] = b.astype(np.float32)
    # [E, k, p, 512] -> [k, p, E, 512]
    w_host = np.ascontiguousarray(
        w_aug.astype(BF16).reshape(NUM_EXPERTS, N_KCH, 128, D_MODEL)
        .transpose(1, 2, 0, 3)).reshape(N_KCH, 128, NUM_EXPERTS * D_MODEL)

    in_maps = []
    for c in range(N_CORES):
        sl = slice(c * S, (c + 1) * S)
        widx = (top_idx[sl].reshape(1, 2 * S) * D_MODEL).astype(np.int32)
        in_maps.append({
            "xab": xab[sl],
            "w": w_host,
            "widx": widx,
        })
    return in_maps


def kernel(cycle_curve_data, logits, moe_masks, W, b):
    if "nc" not in _CACHE:
        _CACHE["nc"] = _build_nc()
    nc = _CACHE["nc"]

    in_maps = _prep_inputs(cycle_curve_data, logits, moe_masks, W, b)

    trace = bool(int(os.environ.get("KERNEL_PROFILE", "0")))
    res = run_bass_kernel_spmd(
        nc, in_maps, core_ids=list(range(N_CORES)), trace=trace
    )
    _CACHE["last_results"] = res

    out = np.empty((B, L, D_MODEL), ml_dtypes.bfloat16)
    for c in range(N_CORES):
        out[c * S: (c + 1) * S] = res.results[c]["y"]
    return out


# revision 18
# speedup vs baseline: 1.0616x; 1.0616x over previous
"""Trainium2 Bass kernel for BatteryMoEFlattenIntraCycleMoELayer.

Computation (reference):
    gates = renorm(top2(softmax(logits) * mask))          # [B, E]
    x = cycle_curve_data.reshape(B, L, 900)
    out[b] = sum_e gates[b,e] * (x[b] @ W[e] + b[e])      # -> bf16 [B, L, 512]

Strategy (bf16, gate-prescaled x, K padded to 1024):
  - Host: compute gates + top-2 routing; build TWO gate-prescaled
    copies of x per sample (xA = gA*x_aug, xB = gB*x_aug, bias row
    included), packed feat-major [B, 128, 8, 128] bf16 (k = sub*128+p,
    zero-padded K 901->1024).  W augmented/padded the same way.
  - Because x carries the gate, both experts' matmuls accumulate into
    ONE PSUM bank per sample: 16 uniform [128,128]x[128,512] bf16
    matmuls -> psum; the combine collapses to a single ACT-engine
    copy/cast psum -> bf16.  One bank/sample makes 8 samples
    k-in-flight possible, which hides the 8.4 MB weight stream during
    phase 1 (k-outer waves of 16 matmuls >= per-tile DMA time).
  - Shard B across 8 cores (64 samples each); routing carried as data
    (per-sample W-slot offsets read into PE registers -> dynamic APs
    on the moving W operand), so one SPMD program serves all cores.
"""

import os
import sys

for _p in ("/opt/trn_rl_repo", "/root/.axon_site/_ro/trn_rl_repo"):
    if os.path.isdir(_p) and _p not in sys.path:
        sys.path.insert(0, _p)

import numpy as np
import ml_dtypes

import concourse.bass as bass
import concourse.mybir as mybir
import concourse.tile as tile
from concourse import bacc
from concourse.bass_utils import run_bass_kernel_spmd
from concourse.bass_values import RuntimeValue

B, L, CURVE_LEN = 512, 128, 300
FEAT = 3 * CURVE_LEN          # 900
FEAT_AUG = FEAT + 1           # 901 (bias row)
K_PAD = 1024                  # zero-padded K: 8 uniform chunks of 128
N_KCH = 8
D_MODEL = 512
NUM_EXPERTS = 8
TOP_K = 2
EPS = 1e-9
N_CORES = 8
S = B // N_CORES              # 64 samples per core

BF16 = ml_dtypes.bfloat16

_CACHE = {}


def _build_nc():
    """Build the SPMD Bass program (routing-independent)."""
    nc = bacc.Bacc(trn_type="TRN2")
    f32 = mybir.dt.float32
    bf16 = mybir.dt.bfloat16
    i32 = mybir.dt.int32

    # gate-prescaled x copies: [S, part, sub, L] bf16, k = sub*128 + part
    xa_h = nc.declare_dram_parameter("xa", [S, 128, N_KCH, L], bf16,
                                     isOutput=False)
    xb_h = nc.declare_dram_parameter("xb", [S, 128, N_KCH, L], bf16,
                                     isOutput=False)
    # w per k-chunk: [k, part, expert*512] bf16 (zero-padded rows)
    w_h = nc.declare_dram_parameter(
        "w", [N_KCH, 128, NUM_EXPERTS * D_MODEL], bf16, isOutput=False)
    widx_h = nc.declare_dram_parameter("widx", [1, 2 * S], i32, isOutput=False)
    y_h = nc.declare_dram_parameter("y", [S, L, D_MODEL], bf16, isOutput=True)

    with tile.TileContext(nc) as tc:
        with (
            tc.tile_pool(name="cpool", bufs=1) as cpool,
            tc.tile_pool(name="xpool", bufs=20) as xpool,
            tc.tile_pool(name="opool", bufs=6) as opool,
            tc.tile_pool(name="pspool", bufs=8, space="PSUM") as pspool,
        ):
            widx_sb = cpool.tile([1, 2 * S], i32)
            nc.sync.dma_start(out=widx_sb[:, :], in_=widx_h[:, :])

            w_sb = []
            for k in range(N_KCH):
                wt = cpool.tile([128, NUM_EXPERTS * D_MODEL], bf16,
                                name=f"w_sb_{k}")
                w_sb.append(wt)

            def load_w(k, nsplit=4):
                # column chunks spread across queues; later tiles use
                # fewer chunks to cut Sync-engine trigger serialization
                WCOL = NUM_EXPERTS * D_MODEL // nsplit
                for c in range(nsplit):
                    nc.sync.dma_start(
                        out=w_sb[k][:, c * WCOL: (c + 1) * WCOL],
                        in_=w_h[k, :, c * WCOL: (c + 1) * WCOL],
                    )

            # ring of PE registers for the per-sample W-slot offsets
            NRING = 16
            wregs = [nc.tensor.alloc_register(f"widx_reg{i}")
                     for i in range(NRING)]
            WMAX = (NUM_EXPERTS - 1) * D_MODEL

            def load_x(s):
                xA = xpool.tile([128, N_KCH, L], bf16, tag="x",
                                name=f"xa_sb_{s}")
                xB = xpool.tile([128, N_KCH, L], bf16, tag="x",
                                name=f"xb_sb_{s}")
                nc.sync.dma_start(out=xA[:, :, :], in_=xa_h[s, :, :, :])
                nc.sync.dma_start(out=xB[:, :, :], in_=xb_h[s, :, :, :])
                return xA, xB

            def load_widx(s0):
                # 8 registers <- widx[2*s0 : 2*s0+8] (4 samples) in one load
                regs = [wregs[(2 * s0 + j) % NRING] for j in range(8)]
                nc.tensor.reg_load(regs, widx_sb[0:1, 2 * s0: 2 * s0 + 8])
                return [RuntimeValue(val=r, min_val=0, max_val=WMAX)
                        for r in regs]

            def mm(ps, x_sb, j, rv, k, start, stop):
                nc.tensor.matmul(
                    ps[:, :], x_sb[j][:, k, :],
                    w_sb[k][:, bass.ds(rv, D_MODEL)],
                    start=start, stop=stop,
                )

            def combine(s, ps):
                o_sb = opool.tile([128, D_MODEL], bf16, tag="o", name=f"o_{s}")
                nc.scalar.copy(o_sb[:, :], ps[:, :])
                nc.sync.dma_start(out=y_h[s, :, :], in_=o_sb[:, :])

            def kouter_group(samples, xs, rv_of):
                """k-outer over a group of samples, 1 PSUM bank each."""
                pss = {s: pspool.tile([128, D_MODEL], f32, tag="ps",
                                      name=f"ps_{s}") for s in samples}
                for k in range(N_KCH):
                    for s in samples:
                        rvA, rvB = rv_of[s]
                        mm(pss[s], xs[s], 0, rvA, k,
                           start=(k == 0), stop=False)
                        mm(pss[s], xs[s], 1, rvB, k,
                           start=False, stop=(k == N_KCH - 1))
                for s in samples:
                    combine(s, pss[s])

            # --- startup DMA order: first group's deps first, W stream
            # interleaved with later groups' x tiles ---
            xs = {}
            load_w(0)
            for s in range(0, 4):
                xs[s] = load_x(s)
            load_w(1)
            for s in range(4, 8):
                xs[s] = load_x(s)
            load_w(2)
            for s in range(8, 12):
                xs[s] = load_x(s)
            load_w(3)
            load_w(4)
            for s in range(12, 16):
                xs[s] = load_x(s)
            load_w(5)
            for s in range(16, 20):
                xs[s] = load_x(s)
            load_w(6)
            load_w(7)

            rv_of = {}

            def load_rv(s0s):
                # each 8-reg batch lands in alternating ring halves; the
                # in-order tensor queue makes reuse safe once the prior
                # group's matmuls have been emitted
                for s0 in s0s:
                    rvs = load_widx(s0)
                    for j in range(4):
                        rv_of[s0 + j] = (rvs[2 * j], rvs[2 * j + 1])

            # --- phase 1: k-outer groups sized to hide the W stream ---
            load_rv((0,))
            kouter_group(range(0, 4), xs, rv_of)
            load_rv((4, 8))
            kouter_group(range(4, 12), xs, rv_of)
            load_rv((12, 16))
            kouter_group(range(12, 20), xs, rv_of)
            for s in range(20):
                del xs[s]

            # --- phase 2: steady state, sample-major ---
            P2 = 20
            for s in range(P2, S):
                xT = load_x(s)
                if s % 4 == 0:
                    rvs = load_widx(s)
                    for j in range(4):
                        if s + j < S:
                            rv_of[s + j] = (rvs[2 * j], rvs[2 * j + 1])
                rvA, rvB = rv_of[s]

                ps = pspool.tile([128, D_MODEL], f32, tag="ps",
                                 name=f"ps2_{s}")
                for k in range(N_KCH):
                    mm(ps, xT, 0, rvA, k, start=(k == 0), stop=False)
                    mm(ps, xT, 1, rvB, k, start=False, stop=(k == N_KCH - 1))
                combine(s, ps)

    nc.finalize()  # Bacc: reg graph-coloring + codegen passes, then freeze
    return nc


def _gates_np(logits, moe_masks):
    """Mirror reference _gates in numpy (fp32)."""
    lg = logits.astype(np.float32)
    m = lg.max(axis=1, keepdims=True)
    e = np.exp(lg - m)
    g = e / e.sum(axis=1, keepdims=True)
    g = g * (moe_masks == 1).astype(np.float32)
    # top-2, ties -> lower index first (matches jax.lax.top_k)
    top_idx = np.argsort(-g, axis=1, kind="stable")[:, :TOP_K]
    rows = np.arange(g.shape[0])[:, None]
    gsel = g[rows, top_idx]                                  # [B, 2]
    gsel = gsel / (gsel.sum(axis=1, keepdims=True) + EPS)
    return gsel.astype(np.float32), top_idx.astype(np.int32)


def _pack_x(xs):
    """[B, L, K_PAD] f32 -> [B, 128, sub, L] bf16 with k = sub*128 + p."""
    return np.ascontiguousarray(
        xs.astype(BF16).reshape(B, L, N_KCH, 128).transpose(0, 3, 2, 1))


def _prep_inputs(cycle_curve_data, logits, moe_masks, W, b):
    gsel, top_idx = _gates_np(logits, moe_masks)

    xf = cycle_curve_data.reshape(B, L, FEAT).astype(np.float32, copy=False)
    xq = np.zeros((B, L, K_PAD), np.float32)
    xq[:, :, :FEAT] = xf
    xq[:, :, FEAT] = 1.0
    xa = _pack_x(xq * gsel[:, 0].reshape(B, 1, 1))
    xb = _pack_x(xq * gsel[:, 1].reshape(B, 1, 1))

    w_aug = np.zeros((NUM_EXPERTS, K_PAD, D_MODEL), np.float32)
    w_aug[:, :FEAT, :] = W.astype(np.float32)
    w_aug[:, FEAT, :] = b.astype(np.float32)
    # [E, k, p, 512] -> [k, p, E, 512]
    w_host = np.ascontiguousarray(
        w_aug.astype(BF16).reshape(NUM_EXPERTS, N_KCH, 128, D_MODEL)
        .transpose(1, 2, 0, 3)).reshape(N_KCH, 128, NUM_EXPERTS * D_MODEL)

    in_maps = []
    for c in range(N_CORES):
        sl = slice(c * S, (c + 1) * S)
        widx = (top_idx[sl].reshape(1, 2 * S) * D_MODEL).astype(np.int32)
        in_maps.append({
            "xa": xa[sl],
            "xb": xb[sl],
            "w": w_host,
            "widx": widx,
        })
    return in_maps


def kernel(cycle_curve_data, logits, moe_masks, W, b):
    if "nc" not in _CACHE:
        _CACHE["nc"] = _build_nc()
    nc = _CACHE["nc"]

    in_maps = _prep_inputs(cycle_curve_data, logits, moe_masks, W, b)

    trace = bool(int(os.environ.get("KERNEL_PROFILE", "0")))
    res = run_bass_kernel_spmd(
        nc, in_maps, core_ids=list(range(N_CORES)), trace=trace
    )
    _CACHE["last_results"] = res

    out = np.empty((B, L, D_MODEL), ml_dtypes.bfloat16)
    for c in range(N_CORES):
        out[c * S: (c + 1) * S] = res.results[c]["y"]
    return out
